# revision 38
# baseline (speedup 1.0000x reference)
"""Causal attention (q/k/v proj + post-softmax-mask renorm attention) on 8
Trainium2 NeuronCores, two SPMD Bass launches.

All large DMA sources are pre-tiled on the host into the exact SBUF tile
layout ("blobs"), so every load is a single ~128-descriptor DMA with
16KB-per-partition contiguous runs (the naive (t p)->p t rearrange costs
~6us of DGE descriptor-push per 2MB and starved the PE).

Launch 1 (uniform, bf16): d_out-sharded QKV projections. Core c computes
qT/kT/vT slices [256, 4096] for its d_out slice; psum f32, outputs bf16.
psum->sbuf copies alternate DVE/ACT; stores alternate GpSimd/Sync queues.

Host: reassembles qT/kT [2048, 4096] / v [4096, 2048], casts q/k to
fp8e4m3 (free), builds launch-2 blobs.

Launch 2 (8 variants via tc.Switch on partition id): causal attention.
Core c handles q-rows [256c, +256) u [256(15-c), +256) (zigzag balance).
Scores via fp8 DoubleRow matmuls (2x PE rate, 8 chained steps over
k=2048), additive -1e9 diagonal mask via DVE on psum, exp (+ fused
row-sums via accum_out) on ACT -> bf16 e-tiles, PE-transpose, AV in bf16
with 8-bank psum accumulation over j in two d-halves; each row-block
drains as soon as its accumulation stops (DVE tensor_scalar for one
d-chunk, ACT Copy-with-scale for the other).

Softmax note: reference computes full softmax then masks + renormalizes;
the full-softmax denominator cancels, so this equals causal softmax
computed directly (exp without max subtraction is safe: |scores/sqrt(d)|
<= ~3 for these input scales). fp8e4m3 q/k gives ~7e-3 max rel err
(emulated) vs the 2e-2 gate.
"""
import os
import numpy as np

import concourse.bacc as bacc
import concourse.mybir as mybir
import concourse.tile as tile
from concourse.bass_utils import run_bass_kernel_spmd

BF16 = mybir.dt.bfloat16
F32 = mybir.dt.float32
FP8 = mybir.dt.float8e4
NPBF = mybir.dt.np(BF16)
NPF8 = mybir.dt.np(FP8)

S, D = 4096, 2048
DS = D // 8            # 256: per-core d_out slice (launch 1)
NT = D // 128          # 16 contraction tiles
SCALE = 1.0 / np.sqrt(D)
USE_FP8 = True         # fp8 DoubleRow scores in launch 2

_cache = {}
last_exec_ns = {}      # filled when BASS_KERNEL_TRACE=1 (test.py)


def _trace_on():
    return os.environ.get("BASS_KERNEL_TRACE", "") == "1"


def _ceil_div(a, b):
    return -(-a // b)


def _build_l1():
    nc = bacc.Bacc("TRN2", target_bir_lowering=False, debug=False)
    # blobs: x [b, tc4, p, t, i], w [tc4, p, t, d]
    d_x = nc.dram_tensor("xb", [4, 4, 128, 4, 1024], BF16,
                         kind="ExternalInput")
    d_wq = nc.dram_tensor("wqb", [4, 128, 4, DS], BF16, kind="ExternalInput")
    d_wk = nc.dram_tensor("wkb", [4, 128, 4, DS], BF16, kind="ExternalInput")
    d_wv = nc.dram_tensor("wvb", [4, 128, 4, DS], BF16, kind="ExternalInput")
    qk_dt = FP8 if USE_FP8 else BF16
    d_qT = nc.dram_tensor("qT", [DS, S], qk_dt, kind="ExternalOutput")
    d_kT = nc.dram_tensor("kT", [DS, S], qk_dt, kind="ExternalOutput")
    d_v = nc.dram_tensor("vT", [DS, S], BF16, kind="ExternalOutput")

    NB = 4
    with tile.TileContext(nc) as tc:
        with (
            tc.tile_pool(name="w", bufs=1) as wp,
            tc.tile_pool(name="xb", bufs=2) as xp,
            tc.tile_pool(name="ob", bufs=6) as op,
            tc.tile_pool(name="pqk", bufs=4, space="PSUM") as pqk,
        ):
            NC = 4
            w_tiles = {}
            for wi in range(3):
                for tc4 in range(NC):
                    wt = wp.tile([128, 4, DS], BF16, tag=f"w{wi}_{tc4}",
                                 name=f"w{wi}_{tc4}")
                    w_tiles[wi, tc4] = wt

            def load_xb(b):
                tiles = []
                for tc4 in range(NC):
                    t_x = xp.tile([128, 4, 1024], BF16, tag=f"xb{tc4}",
                                  name=f"xt{tc4}")
                    if b == 0:
                        # DMA engines drain descriptor FIFOs in push order:
                        # split each tile across both queues, in tc4
                        # priority order, so tile0 completes first
                        nc.sync.dma_start(t_x[:, 0:2, :], d_x[0, tc4, :, 0:2, :])
                        nc.scalar.dma_start(t_x[:, 2:4, :],
                                            d_x[0, tc4, :, 2:4, :])
                    else:
                        xeng = nc.scalar if tc4 % 2 else nc.sync
                        xeng.dma_start(t_x[:], d_x[b, tc4])
                    tiles.append(t_x)
                return tiles

            for tc4 in range(NC):
                nc.gpsimd.dma_start(w_tiles[0, tc4][:], d_wq[tc4])
            xb0 = load_xb(0)
            for tc4 in range(NC):
                e1 = nc.sync if tc4 % 2 == 0 else nc.scalar
                e2 = nc.scalar if tc4 % 2 == 0 else nc.sync
                e1.dma_start(w_tiles[1, tc4][:], d_wk[tc4])
                e2.dma_start(w_tiles[2, tc4][:], d_wv[tc4])

            cnt = [0]
            for b in range(NB):
                x_tiles = xb0 if b == 0 else load_xb(b)
                for wi, d_o in ((0, d_qT), (1, d_kT), (2, d_v)):
                    for dp in range(DS // 128):
                        p_a = pqk.tile([128, 512], F32, tag="p_a")
                        p_b = pqk.tile([128, 512], F32, tag="p_b")
                        for t in range(NT):
                            w_tile = w_tiles[wi, t // 4][:, t % 4,
                                                         dp * 128:(dp + 1) * 128]
                            nc.tensor.matmul(
                                p_a[:], w_tile, x_tiles[t // 4][:, t % 4, 0:512],
                                start=(t == 0), stop=(t == NT - 1))
                            nc.tensor.matmul(
                                p_b[:], w_tile, x_tiles[t // 4][:, t % 4, 512:1024],
                                start=(t == 0), stop=(t == NT - 1))
                        o_dt = qk_dt if wi < 2 else BF16
                        for h, p_h in ((0, p_a), (1, p_b)):
                            t_o = op.tile([128, 512], o_dt,
                                          tag="oqk8" if wi < 2 else "oqkb",
                                          name="t_o")
                            if cnt[0] % 2 == 0:
                                nc.vector.tensor_copy(t_o[:], p_h[:])
                            else:
                                nc.scalar.activation(
                                    t_o[:], p_h[:],
                                    mybir.ActivationFunctionType.Copy)
                            # stores on gpsimd only: a store parked in the
                            # sync/scalar queue would head-of-line block the
                            # next block's x loads
                            nc.gpsimd.dma_start(
                                d_o[dp * 128:(dp + 1) * 128,
                                    b * 1024 + h * 512: b * 1024 + (h + 1) * 512],
                                t_o[:])
                            cnt[0] += 1
    nc.compile()
    return nc


def _build_l2():
    nc = bacc.Bacc("TRN2", target_bir_lowering=False, debug=False)
    qk_dt = FP8 if USE_FP8 else BF16
    # blobs: q [p, t, i], k [ch, p, t, j], v [half, g, p, n, d]
    d_q = nc.dram_tensor("qb", [128, NT, 512], qk_dt, kind="ExternalInput")
    d_k = nc.dram_tensor("kb", [8, 128, NT, 512], qk_dt,
                         kind="ExternalInput")
    d_v = nc.dram_tensor("vb", [2, 8, 128, 4, 1024], BF16,
                         kind="ExternalInput")
    d_mask = nc.dram_tensor("mask", [128, 128], F32, kind="ExternalInput")
    d_eye = nc.dram_tensor("eye", [128, 128], BF16, kind="ExternalInput")
    d_out = nc.dram_tensor("out", [512, D], BF16, kind="ExternalOutput")

    with tile.TileContext(nc) as tc:
        with (
            tc.tile_pool(name="cst", bufs=1) as cst,
            tc.tile_pool(name="qp", bufs=1) as qp,
            tc.tile_pool(name="kc", bufs=8) as kcp,
            tc.tile_pool(name="vc", bufs=4) as vcp,
            tc.tile_pool(name="ec", bufs=4) as ecp,
            tc.tile_pool(name="et", bufs=1) as etp,
            tc.tile_pool(name="sm", bufs=1) as smp,
            tc.tile_pool(name="ob", bufs=6) as obp,
        ):
            t_mask = cst.tile([128, 128], F32, tag="mask")
            t_eye = cst.tile([128, 128], BF16, tag="eye")
            nc.gpsimd.dma_start(t_mask[:], d_mask.ap())
            nc.gpsimd.dma_start(t_eye[:], d_eye.ap())
            # q + first three k chunks load BEFORE the switch: the branch
            # resolution costs ~7us and would otherwise gate these pushes
            t_q = qp.tile([128, NT, 512], qk_dt, tag="qT")
            nc.sync.dma_start(t_q[:, 0:8, :], d_q[:, 0:8, :])
            nc.scalar.dma_start(t_q[:, 8:16, :], d_q[:, 8:16, :])
            # ALL k chunks pre-branch on the SYNC queue only: FIFO descriptor
            # order delivers chunk ch before ch+1, and the scalar queue stays
            # nearly empty so ITS branch resolves fast -- the first exp (and
            # with it the score-psum ring reuse) is gated on scalar's branch
            pre_k = []
            for ch in range(8):
                t_kc = kcp.tile([128, NT, 512], qk_dt, tag="kc", name="t_kc")
                if ch == 0:
                    # tiny t=0..1 slice first: the first DoubleRow step's
                    # only k dependency, ready ~3us before the full chunk
                    nc.sync.dma_start(t_kc[:, 0:2, :], d_k[0, :, 0:2, :])
                    nc.sync.dma_start(t_kc[:, 2:16, :], d_k[0, :, 2:16, :])
                else:
                    nc.sync.dma_start(t_kc[:], d_k[ch])
                pre_k.append(t_kc)

            pid = nc.partition_id()
            for c in tc.Switch(pid, 8):
                lim = [2 * c + 1, 2 * c + 2, 31 - 2 * c, 32 - 2 * c]
                nch = [_ceil_div(l, 4) for l in lim]
                NCH = nch[3]
                LIMX = lim[3]
                t_eT = []
                t_asum = []
                for u in range(4):
                    cap = 16 if u < 2 else 32
                    te = etp.tile([128, cap, 128], BF16, tag=f"eT{u}",
                                  name=f"eT{u}")
                    t_eT.append(te)
                    ta = smp.tile([128, 8], F32, tag=f"asum{u}",
                                  name=f"asum{u}")
                    t_asum.append(ta)

                # ---- phase 1: scores -> exp(+rowsums) -> transpose ----
                ph1 = tc.tile_pool(name=f"ps{c}", bufs=6, space="PSUM")
                psp = ph1.__enter__()
                ph1t = tc.tile_pool(name=f"pt{c}", bufs=2, space="PSUM")
                ptp = ph1t.__enter__()
                for ch in range(NCH):
                    t_kc = pre_k[ch]
                    pend = []

                    def flush_pend():
                        u2, jw2, t_e2, ch2 = pend.pop(0)
                        for qq in range(_ceil_div(jw2, 128)):
                            p_t = ptp.tile([128, 128], BF16, tag="p_t")
                            nc.tensor.transpose(
                                p_t[:], t_e2[:, qq * 128:(qq + 1) * 128],
                                t_eye[:])
                            nc.vector.tensor_copy(
                                t_eT[u2][:, ch2 * 4 + qq, :], p_t[:])

                    for u in range(4):
                        if ch >= nch[u]:
                            continue
                        jw = min(512, lim[u] * 128 - ch * 512)
                        p_s = psp.tile([128, 512], F32, tag="p_s")
                        if USE_FP8:
                            for t2 in range(NT // 2):
                                nc.tensor.matmul(
                                    p_s[:, :jw],
                                    t_q[:, 2 * t2:2 * t2 + 2,
                                        u * 128:(u + 1) * 128],
                                    t_kc[:, 2 * t2:2 * t2 + 2, :jw],
                                    start=(t2 == 0), stop=(t2 == NT // 2 - 1),
                                    perf_mode=mybir.MatmulPerfMode.DoubleRow)
                        else:
                            for t in range(NT):
                                nc.tensor.matmul(
                                    p_s[:, :jw],
                                    t_q[:, t, u * 128:(u + 1) * 128],
                                    t_kc[:, t, :jw],
                                    start=(t == 0), stop=(t == NT - 1))
                        if ch == (lim[u] - 1) // 4:
                            off = ((lim[u] - 1) % 4) * 128
                            nc.vector.tensor_add(
                                p_s[:, off:off + 128],
                                p_s[:, off:off + 128],
                                t_mask[:])
                        t_e = ecp.tile([128, 512], BF16, tag="t_e")
                        nc.scalar.activation(
                            t_e[:, :jw], p_s[:, :jw],
                            mybir.ActivationFunctionType.Exp,
                            scale=SCALE,
                            accum_out=t_asum[u][:, ch:ch + 1])
                        pend.append((u, jw, t_e, ch))
                        if len(pend) > 1:
                            flush_pend()
                    while pend:
                        flush_pend()

                t_recips = []
                for u in range(4):
                    t_sum = smp.tile([128, 1], F32, tag=f"sum{u}",
                                     name=f"sum{u}")
                    nc.vector.reduce_sum(
                        t_sum[:], t_asum[u][:, :nch[u]],
                        axis=mybir.AxisListType.X)
                    t_rc = smp.tile([128, 1], F32, tag=f"recip{u}",
                                    name=f"recip{u}")
                    nc.vector.reciprocal(t_rc[:], t_sum[:])
                    t_recips.append(t_rc)
                ph1t.__exit__(None, None, None)
                ph1.__exit__(None, None, None)

                # ---- phase 2: AV over j, two d-halves, 8-bank psum ----
                ph2 = tc.tile_pool(name=f"po{c}", bufs=1, space="PSUM")
                pop = ph2.__enter__()
                for half in range(2):

                    def drain_u(u, p_out):
                        for db in range(2):
                            t_o = obp.tile([128, 512], BF16, tag="t_o")
                            if db == 0:
                                nc.vector.tensor_scalar_mul(
                                    t_o[:], p_out[u, db][:], t_recips[u][:])
                            else:
                                nc.scalar.activation(
                                    t_o[:], p_out[u, db][:],
                                    mybir.ActivationFunctionType.Copy,
                                    scale=t_recips[u][:])
                            # gpsimd only: keep stores out of the v-load queues
                            nc.gpsimd.dma_start(
                                d_out[u * 128:(u + 1) * 128,
                                      half * 1024 + db * 512:
                                      half * 1024 + (db + 1) * 512],
                                t_o[:])

                    p_out = {}
                    for u in range(4):
                        for db in range(2):
                            p_o = pop.tile([128, 512], F32, tag=f"po{u}{db}",
                                           name=f"po{u}{db}")
                            p_out[u, db] = p_o
                    t_vc = None
                    for jt in range(LIMX):
                        jtm = jt % 4
                        if jtm == 0:
                            g = jt // 4
                            t_vc = vcp.tile([128, 4, 1024], BF16, tag="t_vc")
                            veng = nc.scalar if g % 2 else nc.sync
                            veng.dma_start(t_vc[:], d_v[half, g])
                        for u in range(4):
                            if jt >= lim[u]:
                                continue
                            for db in range(2):
                                nc.tensor.matmul(
                                    p_out[u, db][:],
                                    t_eT[u][:, jt, :],
                                    t_vc[:, jtm, db * 512:(db + 1) * 512],
                                    start=(jt == 0),
                                    stop=(jt == lim[u] - 1))
                            if jt == lim[u] - 1:
                                drain_u(u, p_out)
                ph2.__exit__(None, None, None)
    nc.compile()
    return nc


def kernel(x, W_q, W_k, W_v):
    x = np.asarray(x, dtype=np.float32)
    W_q = np.asarray(W_q, dtype=np.float32)
    W_k = np.asarray(W_k, dtype=np.float32)
    W_v = np.asarray(W_v, dtype=np.float32)
    if "l1" not in _cache:
        _cache["l1"] = _build_l1()
    if "l2" not in _cache:
        _cache["l2"] = _build_l2()
    nc1, nc2 = _cache["l1"], _cache["l2"]
    trace = _trace_on()

    # ---- launch 1: QKV projections ----
    xT = np.ascontiguousarray(x.T).astype(NPBF)
    # x blob [b, tc4, p, t, i]
    xb = np.ascontiguousarray(
        xT.reshape(4, 4, 128, 4, 1024).transpose(3, 0, 2, 1, 4))
    in_maps = []
    for c in range(8):
        sl = slice(c * DS, (c + 1) * DS)
        im = {"xb": xb}
        for nm, W in (("wqb", W_q), ("wkb", W_k), ("wvb", W_v)):
            WT = np.ascontiguousarray(W.T[:, sl]).astype(NPBF)  # [2048, 256]
            im[nm] = np.ascontiguousarray(
                WT.reshape(4, 4, 128, DS).transpose(0, 2, 1, 3))
        in_maps.append(im)
    res1 = run_bass_kernel_spmd(nc1, in_maps, core_ids=list(range(8)),
                                trace=trace)
    qT = np.vstack([res1.results[c]["qT"] for c in range(8)])
    kT = np.vstack([res1.results[c]["kT"] for c in range(8)])
    v = np.vstack([res1.results[c]["vT"] for c in range(8)]).T

    # ---- launch 2: causal attention ----
    npqk = NPF8 if USE_FP8 else NPBF
    kb = np.ascontiguousarray(
        kT.astype(npqk).reshape(16, 128, 8, 512).transpose(2, 1, 0, 3))
    vb = np.ascontiguousarray(
        v.astype(NPBF).reshape(8, 4, 128, 2, 1024).transpose(3, 0, 2, 1, 4))
    ii = np.arange(128)[:, None]
    jj = np.arange(128)[None, :]
    mask = np.where(jj <= ii, 0.0, -1e9).astype(np.float32)
    eye = np.eye(128, dtype=NPBF)
    in_maps2 = []
    for c in range(8):
        lo, hi = 256 * c, 256 * (15 - c)
        q_own = np.concatenate([qT[:, lo:lo + 256], qT[:, hi:hi + 256]],
                               axis=1).astype(npqk)
        qb = np.ascontiguousarray(
            q_own.reshape(16, 128, 512).transpose(1, 0, 2))
        in_maps2.append({
            "qb": qb, "kb": kb, "vb": vb, "mask": mask, "eye": eye,
        })
    res2 = run_bass_kernel_spmd(nc2, in_maps2, core_ids=list(range(8)),
                                trace=trace)
    out = np.empty((S, D), np.float32)
    for c in range(8):
        lo, hi = 256 * c, 256 * (15 - c)
        ob = res2.results[c]["out"].astype(np.float32)
        out[lo:lo + 256] = ob[0:256]
        out[hi:hi + 256] = ob[256:512]

    if trace:
        last_exec_ns["l1"] = res1.exec_time_ns
        last_exec_ns["l2"] = res2.exec_time_ns
        last_exec_ns["res1"] = res1
        last_exec_ns["res2"] = res2
    return out
